# revision 13
# baseline (speedup 1.0000x reference)
"""Multi-head attention block (QKV linear -> softmax attention -> proj linear)
for Trainium2, SPMD over 8 NeuronCores.

Sharding: 8 shards = batch (4) x head-group (2 groups of 6 heads).
Each core computes, for its (b, g):
    qkv   = x[b] @ Wqkv[:, cols(g)]            (bf16 matmul, fp32 accum)
    S^T_h = K_h Q_h^T   per head               (keys on partitions)
    P^T_h = exp(SCALE * S^T_h)                 (ACT engine, bf16 out)
    out_h = (P_h @ [V_h | 1]) -> normalize rows by the ones-column sum
    y_g   = concat_h(out_h) @ Wproj[rows(g), :]    (partial, fp32 out)
Host sums the two head-group partials per batch and adds bproj.

Shapes hardcoded: x [4, 2048, 768], Wqkv [768, 2304], Wproj [768, 768].
"""

import os
from contextlib import ExitStack

import numpy as np
import ml_dtypes

import concourse.bass as bass
import concourse.mybir as mybir
import concourse.tile as tile
from concourse import bacc
from concourse.bass_utils import run_bass_kernel_spmd
from concourse.masks import make_identity

B, N, C = 4, 2048, 768
H, D = 12, 64          # total heads, head dim
G = 2                  # head groups (tensor-parallel axis)
HL = H // G            # heads per core = 6
SCALE = D ** -0.5
P = 128
CB = C // P            # 6 contraction blocks
NT = N // P            # 16 row tiles
EG = HL * D            # 384 = per-group width of Q / K / V
NCORES = 8

f32 = mybir.dt.float32
bf16 = mybir.dt.bfloat16

# knobs
PT_BUFS = int(os.environ.get("KRN_PT_BUFS", "25"))
PV_SPLIT = int(os.environ.get("KRN_PV_SPLIT", "1"))  # split-K pieces for PV


def _build_program():
    nc = bacc.Bacc("TRN2", target_bir_lowering=False, debug=False)

    xT = nc.dram_tensor("xT", [C, N], bf16, kind="ExternalInput")           # x[b].T
    wqkv = nc.dram_tensor("wqkv", [C, 3 * EG], bf16, kind="ExternalInput")  # [Qg|Kg|Vg]
    wproj = nc.dram_tensor("wproj", [EG, C], bf16, kind="ExternalInput")    # group rows
    yT = nc.dram_tensor("yT", [C, N], f32, kind="ExternalOutput")           # partial out^T

    with tile.TileContext(nc) as tc, ExitStack() as ctx:
        persist = ctx.enter_context(tc.tile_pool(name="persist", bufs=1))
        ptpool = ctx.enter_context(tc.tile_pool(name="ptpool", bufs=PT_BUFS))
        stpool = ctx.enter_context(tc.tile_pool(name="stpool", bufs=2))
        rpool = ctx.enter_context(tc.tile_pool(name="rpool", bufs=8))
        ypool = ctx.enter_context(tc.tile_pool(name="ypool", bufs=2))
        ps_score = ctx.enter_context(tc.tile_pool(name="ps_score", bufs=3, space="PSUM"))
        ps_small = ctx.enter_context(tc.tile_pool(name="ps_small", bufs=2, space="PSUM"))

        identity = persist.tile([P, P], bf16, tag="identity")
        make_identity(nc, identity)

        # ---- loads ----
        # x^T lives in six "pt"-tagged tiles: slots recycle into P^T buffers
        # once the V phase is done.
        xts = []
        for cb in range(CB):
            xt_cb = ptpool.tile([P, N], bf16, tag="pt", name=f"xt{cb}")
            nc.sync.dma_start(xt_cb[:], xT[cb * P : (cb + 1) * P, :])
            xts.append(xt_cb)
        wq_sb = persist.tile([P, CB, 3 * EG], bf16, tag="wq")
        nc.sync.dma_start(wq_sb[:], wqkv[:].rearrange("(cb p) e -> p cb e", p=P))
        wp_sb = persist.tile([P, EG // P, C], bf16, tag="wp")
        nc.sync.dma_start(wp_sb[:], wproj[:].rearrange("(cb p) c -> p cb c", p=P))

        # persistent attention tiles
        qkT_sb = persist.tile([P, 2 * EG // P, N], bf16, tag="qkT")
        vp_sb = persist.tile([P, NT, HL * (D + 1)], bf16, tag="vp")
        vp4 = vp_sb.rearrange("p m (h c) -> p m h c", c=D + 1)
        nc.vector.memset(vp4[:, :, :, D : D + 1], 1.0)
        og_sb = persist.tile([P, NT, EG], bf16, tag="og")   # heads out [n, ch]

        def emit_qk(eb):
            # Q^T / K^T: out[e128, n512] = Wg[:, eb].T @ x^T
            for nch in range(N // 512):
                qpsum = ps_small.tile([P, 512], f32, tag="sm")
                for cb in range(CB):
                    nc.tensor.matmul(
                        qpsum,
                        wq_sb[:, cb, eb * P : (eb + 1) * P],
                        xts[cb][:, nch * 512 : (nch + 1) * 512],
                        start=(cb == 0),
                        stop=(cb == CB - 1),
                    )
                nc.vector.tensor_copy(
                    qkT_sb[:, eb, nch * 512 : (nch + 1) * 512], qpsum
                )

        def emit_v():
            # V: out[m128, 384] = x^T[:, mt].T @ Wv; scatter into V' (65-stride)
            for mt in range(NT):
                vpsum = ps_small.tile([P, 512], f32, tag="sm")
                for cb in range(CB):
                    nc.tensor.matmul(
                        vpsum[:, :EG],
                        xts[cb][:, mt * P : (mt + 1) * P],
                        wq_sb[:, cb, 2 * EG : 3 * EG],
                        start=(cb == 0),
                        stop=(cb == CB - 1),
                    )
                nc.vector.tensor_copy(
                    vp4[:, mt, :, :D],
                    vpsum[:, :EG].rearrange("p (h d) -> p h d", d=D),
                )

        def emit_scores(h, chaser=None):
            """Scores + exp for one head; returns the P^T tiles."""
            prow = (h % 2) * D
            qblk = h // 2
            kblk = 3 + h // 2
            pts = []
            for mt in range(NT):
                pt = ptpool.tile([P, N], bf16, tag="pt")
                pts.append(pt)
                lhsT = qkT_sb[prow : prow + D, kblk, mt * P : (mt + 1) * P]
                for nch in range(2):
                    spsum = ps_score.tile([P, 1024], f32)
                    for sub in range(2):
                        off = nch * 1024 + sub * 512
                        nc.tensor.matmul(
                            spsum[:, sub * 512 : (sub + 1) * 512],
                            lhsT,
                            qkT_sb[prow : prow + D, qblk, off : off + 512],
                            start=True,
                            stop=True,
                        )
                    nc.scalar.activation(
                        pt[:, nch * 1024 : (nch + 1) * 1024],
                        spsum,
                        mybir.ActivationFunctionType.Exp,
                        scale=SCALE,
                    )
                if chaser is not None:
                    chaser(mt)
            return pts

        def emit_pv_chunk(h, pts, stage, nch):
            """out^T[65, n-chunk] = [V_h | 1]^T @ P_h^T, V' stationary."""
            ovpsum = ps_small.tile([P, 512], f32, tag="sm")
            for mt in range(NT):
                nc.tensor.matmul(
                    ovpsum[: D + 1, :],
                    vp_sb[:, mt, h * (D + 1) : (h + 1) * (D + 1)],
                    pts[mt][:, nch * 512 : (nch + 1) * 512],
                    start=(mt == 0),
                    stop=(mt == NT - 1),
                )
            nc.vector.tensor_copy(
                stage[: D + 1, nch * 512 : (nch + 1) * 512], ovpsum[: D + 1, :]
            )

        def emit_norm(h, stage):
            """Transpose each [65, 128] block of out^T back to [n, 65] and
            divide the 64 out-channels by the ones-column sum."""
            for nt in range(NT):
                tpsum = ps_small.tile([P, 512], bf16, tag="sm")
                nc.tensor.transpose(
                    tpsum[:, : D + 1],
                    stage[: D + 1, nt * P : (nt + 1) * P],
                    identity[: D + 1, : D + 1],
                )
                r = rpool.tile([P, 1], f32)
                nc.vector.reciprocal(r, tpsum[:, D : D + 1])
                nc.vector.tensor_scalar(
                    og_sb[:, nt, h * D : (h + 1) * D],
                    tpsum[:, :D],
                    r,
                    None,
                    mybir.AluOpType.mult,
                )

        def pv_chaser(h, pts, stage):
            # ride along the next head's scores: one PV chunk every 4 m-tiles,
            # then the normalize pass
            def chase(mt):
                if mt % 4 == 3:
                    emit_pv_chunk(h, pts, stage, mt // 4)
                if mt == NT - 1:
                    emit_norm(h, stage)
            return chase

        # ---- emission schedule ----
        emit_qk(3)
        emit_qk(0)
        all_pts = [emit_scores(0)]
        all_stage = [stpool.tile([P, N], bf16, tag="stg", name="stage0")]
        emit_qk(4)
        emit_qk(1)
        emit_qk(5)
        emit_qk(2)
        emit_v()
        for h in range(1, HL):
            stage = stpool.tile([P, N], bf16, tag="stg", name=f"stage{h}")
            all_stage.append(stage)
            all_pts.append(
                emit_scores(h, chaser=pv_chaser(h - 1, all_pts[h - 1], all_stage[h - 1]))
            )
        for nch in range(4):
            emit_pv_chunk(HL - 1, all_pts[HL - 1], all_stage[HL - 1], nch)
        emit_norm(HL - 1, all_stage[HL - 1])

        # ---- transpose heads-out to [ch, n] for proj ----
        ogT_sb = persist.tile([P, EG // P, N], bf16, tag="ogT")
        for nt in range(NT):
            for cb in range(EG // P):
                tpsum2 = ps_small.tile([P, 512], bf16, tag="sm")
                nc.tensor.transpose(
                    tpsum2[:, :P], og_sb[:, nt, cb * P : (cb + 1) * P], identity
                )
                nc.vector.tensor_copy(
                    ogT_sb[:, cb, nt * P : (nt + 1) * P], tpsum2[:, :P]
                )

        # ---- proj, y^T orientation: y^T[cout, n] = Wp^T @ heads-out^T ----
        for coutb in range(CB):
            for nch in range(N // 512):
                ppsum = ps_small.tile([P, 512], f32, tag="sm")
                for cb in range(EG // P):
                    nc.tensor.matmul(
                        ppsum,
                        wp_sb[:, cb, coutb * P : (coutb + 1) * P],
                        ogT_sb[:, cb, nch * 512 : (nch + 1) * 512],
                        start=(cb == 0),
                        stop=(cb == EG // P - 1),
                    )
                y_sb = ypool.tile([P, 512], f32)
                nc.vector.tensor_copy(y_sb, ppsum)
                nc.sync.dma_start(
                    yT[coutb * P : (coutb + 1) * P, nch * 512 : (nch + 1) * 512],
                    y_sb,
                )

    nc.compile()
    return nc


_PROGRAM = None


def _get_program():
    global _PROGRAM
    if _PROGRAM is None:
        _PROGRAM = _build_program()
    return _PROGRAM


def _shard_inputs(x, Wqkv, Wproj):
    bf = ml_dtypes.bfloat16
    in_maps = []
    for core in range(NCORES):
        b, g = core // G, core % G
        xT = np.ascontiguousarray(x[b].T).astype(bf)
        wg = np.concatenate(
            [
                Wqkv[:, g * EG : (g + 1) * EG],
                Wqkv[:, C + g * EG : C + (g + 1) * EG],
                Wqkv[:, 2 * C + g * EG : 2 * C + (g + 1) * EG],
            ],
            axis=1,
        ).astype(bf)
        wp = np.ascontiguousarray(Wproj[g * EG : (g + 1) * EG, :]).astype(bf)
        in_maps.append({"xT": xT, "wqkv": wg, "wproj": wp})
    return in_maps


def _run(x, Wqkv, Wproj, bproj, trace=False):
    nc = _get_program()
    in_maps = _shard_inputs(x, Wqkv, Wproj)
    res = run_bass_kernel_spmd(nc, in_maps, list(range(NCORES)), trace=trace)
    out = np.empty((B, N, C), np.float32)
    for b in range(B):
        out[b] = (res.results[b * G]["yT"] + res.results[b * G + 1]["yT"]).T + bproj
    return out, res


def kernel(x, Wqkv, Wproj, bproj):
    x = np.asarray(x, np.float32)
    Wqkv = np.asarray(Wqkv, np.float32)
    Wproj = np.asarray(Wproj, np.float32)
    bproj = np.asarray(bproj, np.float32)
    out, _ = _run(x, Wqkv, Wproj, bproj)
    return out


# revision 14
# speedup vs baseline: 1.0128x; 1.0128x over previous
"""Multi-head attention block (QKV linear -> softmax attention -> proj linear)
for Trainium2, SPMD over 8 NeuronCores.

Sharding: 8 shards = batch (4) x head-group (2 groups of 6 heads).
Each core computes, for its (b, g):
    qkv   = x[b] @ Wqkv[:, cols(g)]            (bf16 matmul, fp32 accum)
    S^T_h = K_h Q_h^T   per head               (keys on partitions)
    P^T_h = exp(SCALE * S^T_h)                 (ACT engine, bf16 out)
    out_h = (P_h @ [V_h | 1]) -> normalize rows by the ones-column sum
    y_g   = concat_h(out_h) @ Wproj[rows(g), :]    (partial, fp32 out)
Host sums the two head-group partials per batch and adds bproj.

Shapes hardcoded: x [4, 2048, 768], Wqkv [768, 2304], Wproj [768, 768].
"""

import os
from contextlib import ExitStack

import numpy as np
import ml_dtypes

import concourse.bass as bass
import concourse.mybir as mybir
import concourse.tile as tile
from concourse import bacc
from concourse.bass_utils import run_bass_kernel_spmd
from concourse.masks import make_identity

B, N, C = 4, 2048, 768
H, D = 12, 64          # total heads, head dim
G = 2                  # head groups (tensor-parallel axis)
HL = H // G            # heads per core = 6
SCALE = D ** -0.5
P = 128
CB = C // P            # 6 contraction blocks
NT = N // P            # 16 row tiles
EG = HL * D            # 384 = per-group width of Q / K / V
NCORES = 8

f32 = mybir.dt.float32
bf16 = mybir.dt.bfloat16

# knobs
PT_BUFS = int(os.environ.get("KRN_PT_BUFS", "25"))
PV_SPLIT = int(os.environ.get("KRN_PV_SPLIT", "1"))  # split-K pieces for PV


def _build_program():
    nc = bacc.Bacc("TRN2", target_bir_lowering=False, debug=False)

    xT = nc.dram_tensor("xT", [C, N], bf16, kind="ExternalInput")           # x[b].T
    wqkv = nc.dram_tensor("wqkv", [C, 3 * EG], bf16, kind="ExternalInput")  # [Qg|Kg|Vg]
    wproj = nc.dram_tensor("wproj", [EG, C], bf16, kind="ExternalInput")    # group rows
    yT = nc.dram_tensor("yT", [C, N], f32, kind="ExternalOutput")           # partial out^T

    with tile.TileContext(nc) as tc, ExitStack() as ctx:
        persist = ctx.enter_context(tc.tile_pool(name="persist", bufs=1))
        ptpool = ctx.enter_context(tc.tile_pool(name="ptpool", bufs=PT_BUFS))
        stpool = ctx.enter_context(tc.tile_pool(name="stpool", bufs=2))
        rpool = ctx.enter_context(tc.tile_pool(name="rpool", bufs=8))
        ypool = ctx.enter_context(tc.tile_pool(name="ypool", bufs=2))
        ps_score = ctx.enter_context(tc.tile_pool(name="ps_score", bufs=3, space="PSUM"))
        ps_small = ctx.enter_context(tc.tile_pool(name="ps_small", bufs=2, space="PSUM"))

        identity = persist.tile([P, P], bf16, tag="identity")
        make_identity(nc, identity)

        # ---- loads ----
        # x^T lives in six "pt"-tagged tiles: slots recycle into P^T buffers
        # once the V phase is done.
        xts = []
        for cb in range(CB):
            xt_cb = ptpool.tile([P, N], bf16, tag="pt", name=f"xt{cb}")
            nc.sync.dma_start(xt_cb[:], xT[cb * P : (cb + 1) * P, :])
            xts.append(xt_cb)
        wq_sb = persist.tile([P, CB, 3 * EG], bf16, tag="wq")
        nc.sync.dma_start(wq_sb[:], wqkv[:].rearrange("(cb p) e -> p cb e", p=P))
        wp_sb = persist.tile([P, EG // P, C], bf16, tag="wp")
        nc.sync.dma_start(wp_sb[:], wproj[:].rearrange("(cb p) c -> p cb c", p=P))

        # persistent attention tiles
        qkT_sb = persist.tile([P, 2 * EG // P, N], bf16, tag="qkT")
        vp_sb = persist.tile([P, NT, HL * (D + 1)], bf16, tag="vp")
        vp4 = vp_sb.rearrange("p m (h c) -> p m h c", c=D + 1)
        nc.vector.memset(vp4[:, :, :, D : D + 1], 1.0)
        og_sb = persist.tile([P, NT, EG], bf16, tag="og")   # heads out [n, ch]

        def emit_qk(eb):
            # Q^T / K^T: out[e128, n512] = Wg[:, eb].T @ x^T
            for nch in range(N // 512):
                qpsum = ps_small.tile([P, 512], f32, tag="sm")
                for cb in range(CB):
                    nc.tensor.matmul(
                        qpsum,
                        wq_sb[:, cb, eb * P : (eb + 1) * P],
                        xts[cb][:, nch * 512 : (nch + 1) * 512],
                        start=(cb == 0),
                        stop=(cb == CB - 1),
                    )
                nc.vector.tensor_copy(
                    qkT_sb[:, eb, nch * 512 : (nch + 1) * 512], qpsum
                )

        def emit_v():
            # V: out[m128, 384] = x^T[:, mt].T @ Wv; scatter into V' (65-stride)
            for mt in range(NT):
                vpsum = ps_small.tile([P, 512], f32, tag="sm")
                for cb in range(CB):
                    nc.tensor.matmul(
                        vpsum[:, :EG],
                        xts[cb][:, mt * P : (mt + 1) * P],
                        wq_sb[:, cb, 2 * EG : 3 * EG],
                        start=(cb == 0),
                        stop=(cb == CB - 1),
                    )
                nc.vector.tensor_copy(
                    vp4[:, mt, :, :D],
                    vpsum[:, :EG].rearrange("p (h d) -> p h d", d=D),
                )

        def emit_scores(h, chaser=None):
            """Scores + exp for one head; returns the P^T tiles."""
            prow = (h % 2) * D
            qblk = h // 2
            kblk = 3 + h // 2
            pts = []
            for mt in range(NT):
                pt = ptpool.tile([P, N], bf16, tag="pt")
                pts.append(pt)
                lhsT = qkT_sb[prow : prow + D, kblk, mt * P : (mt + 1) * P]
                for nch in range(2):
                    spsum = ps_score.tile([P, 1024], f32)
                    for sub in range(2):
                        off = nch * 1024 + sub * 512
                        nc.tensor.matmul(
                            spsum[:, sub * 512 : (sub + 1) * 512],
                            lhsT,
                            qkT_sb[prow : prow + D, qblk, off : off + 512],
                            start=True,
                            stop=True,
                        )
                    nc.scalar.activation(
                        pt[:, nch * 1024 : (nch + 1) * 1024],
                        spsum,
                        mybir.ActivationFunctionType.Exp,
                        scale=SCALE,
                    )
                if chaser is not None:
                    chaser(mt)
            return pts

        def emit_norm_one(h, stage, nt):
            """Transpose one [65, 128] block of out^T back to [n, 65] and
            divide the 64 out-channels by the ones-column sum."""
            tpsum = ps_small.tile([P, 512], bf16, tag="sm")
            nc.tensor.transpose(
                tpsum[:, : D + 1],
                stage[: D + 1, nt * P : (nt + 1) * P],
                identity[: D + 1, : D + 1],
            )
            r = rpool.tile([P, 1], f32)
            nc.vector.reciprocal(r, tpsum[:, D : D + 1])
            nc.vector.tensor_scalar(
                og_sb[:, nt, h * D : (h + 1) * D],
                tpsum[:, :D],
                r,
                None,
                mybir.AluOpType.mult,
            )

        def pv_chaser(h, pts, stage):
            """Emit PV (out^T = V'^T @ P^T, V' stationary) in 4-matmul pieces
            plus one norm-transpose per step so no monolithic block ever
            head-of-line-blocks the next head's scores on the PE queue.
            Steps 0..15 ride the next head's scores loop; 16..20 drain."""
            st = {}

            def chase(s):
                if s < NT:
                    cn, piece = s // 4, s % 4
                    if piece == 0:
                        st["ps"] = ps_small.tile(
                            [P, 512], f32, tag="sm", name=f"ovps{h}_{cn}"
                        )
                    for mt in range(4 * piece, 4 * piece + 4):
                        nc.tensor.matmul(
                            st["ps"][: D + 1, :],
                            vp_sb[:, mt, h * (D + 1) : (h + 1) * (D + 1)],
                            pts[mt][:, cn * 512 : (cn + 1) * 512],
                            start=(mt == 0),
                            stop=(mt == NT - 1),
                        )
                    if piece == 3:
                        nc.vector.tensor_copy(
                            stage[: D + 1, cn * 512 : (cn + 1) * 512],
                            st["ps"][: D + 1, :],
                        )
                if 5 <= s < 5 + NT:
                    emit_norm_one(h, stage, s - 5)

            return chase

        # ---- emission schedule ----
        emit_qk(3)
        emit_qk(0)
        all_pts = [emit_scores(0)]
        all_stage = [stpool.tile([P, N], bf16, tag="stg", name="stage0")]
        emit_qk(4)
        emit_qk(1)
        emit_qk(5)
        emit_qk(2)
        emit_v()
        for h in range(1, HL):
            stage = stpool.tile([P, N], bf16, tag="stg", name=f"stage{h}")
            all_stage.append(stage)
            ch = pv_chaser(h - 1, all_pts[h - 1], all_stage[h - 1])
            all_pts.append(emit_scores(h, chaser=ch))
            for s in range(NT, NT + 5):
                ch(s)
        ch = pv_chaser(HL - 1, all_pts[HL - 1], all_stage[HL - 1])
        for s in range(NT + 5):
            ch(s)

        # ---- transpose heads-out to [ch, n] for proj ----
        ogT_sb = persist.tile([P, EG // P, N], bf16, tag="ogT")
        for nt in range(NT):
            for cb in range(EG // P):
                tpsum2 = ps_small.tile([P, 512], bf16, tag="sm")
                nc.tensor.transpose(
                    tpsum2[:, :P], og_sb[:, nt, cb * P : (cb + 1) * P], identity
                )
                nc.vector.tensor_copy(
                    ogT_sb[:, cb, nt * P : (nt + 1) * P], tpsum2[:, :P]
                )

        # ---- proj, y^T orientation: y^T[cout, n] = Wp^T @ heads-out^T ----
        for coutb in range(CB):
            for nch in range(N // 512):
                ppsum = ps_small.tile([P, 512], f32, tag="sm")
                for cb in range(EG // P):
                    nc.tensor.matmul(
                        ppsum,
                        wp_sb[:, cb, coutb * P : (coutb + 1) * P],
                        ogT_sb[:, cb, nch * 512 : (nch + 1) * 512],
                        start=(cb == 0),
                        stop=(cb == EG // P - 1),
                    )
                y_sb = ypool.tile([P, 512], f32)
                nc.vector.tensor_copy(y_sb, ppsum)
                nc.sync.dma_start(
                    yT[coutb * P : (coutb + 1) * P, nch * 512 : (nch + 1) * 512],
                    y_sb,
                )

    nc.compile()
    return nc


_PROGRAM = None


def _get_program():
    global _PROGRAM
    if _PROGRAM is None:
        _PROGRAM = _build_program()
    return _PROGRAM


def _shard_inputs(x, Wqkv, Wproj):
    bf = ml_dtypes.bfloat16
    in_maps = []
    for core in range(NCORES):
        b, g = core // G, core % G
        xT = np.ascontiguousarray(x[b].T).astype(bf)
        wg = np.concatenate(
            [
                Wqkv[:, g * EG : (g + 1) * EG],
                Wqkv[:, C + g * EG : C + (g + 1) * EG],
                Wqkv[:, 2 * C + g * EG : 2 * C + (g + 1) * EG],
            ],
            axis=1,
        ).astype(bf)
        wp = np.ascontiguousarray(Wproj[g * EG : (g + 1) * EG, :]).astype(bf)
        in_maps.append({"xT": xT, "wqkv": wg, "wproj": wp})
    return in_maps


def _run(x, Wqkv, Wproj, bproj, trace=False):
    nc = _get_program()
    in_maps = _shard_inputs(x, Wqkv, Wproj)
    res = run_bass_kernel_spmd(nc, in_maps, list(range(NCORES)), trace=trace)
    out = np.empty((B, N, C), np.float32)
    for b in range(B):
        out[b] = (res.results[b * G]["yT"] + res.results[b * G + 1]["yT"]).T + bproj
    return out, res


def kernel(x, Wqkv, Wproj, bproj):
    x = np.asarray(x, np.float32)
    Wqkv = np.asarray(Wqkv, np.float32)
    Wproj = np.asarray(Wproj, np.float32)
    bproj = np.asarray(bproj, np.float32)
    out, _ = _run(x, Wqkv, Wproj, bproj)
    return out
